# revision 5
# baseline (speedup 1.0000x reference)
"""Trainium2 Bass kernel for a 3-layer FCL + size-5 sliding-window stack.

Reference computation (fp32):
    h = relu(x @ W1.T)          # [N, 10]
    t = relu(h @ W2.T + b2)     # [N, 5]
    out[n] = concat(t[n-2..n+2])  zero-padded  -> [N, 25]

Strategy (8 cores, data-parallel over rows, halo recomputed per core):
  - Host prep is layout-only: x is cast to bf16 and pre-transposed so each
    core receives xT [320, 25088] (25000 own rows + 2-row halo each side,
    zero padded).  This halves the HBM x read (16MB vs 32MB fp32) and puts
    the 320-dim contraction directly on partitions - the tensor engine
    never has to transpose anything.
  - The whole pipeline runs in the transposed layout.  The stationary
    operands are column-replicated (W1T x12 -> [*,120], W2T x25 -> [10,125])
    because M is free on the systolic array (cost is N streaming cycles
    only): the ~97% array utilization keeps the PE HAM clock-gate at
    2.4 GHz (thin M=10 matmuls never un-throttle it and run at 1.2 GHz),
    and the 25 t.T copies land on 125 partitions so the output stores
    read from ~7 SBUF AXI ports instead of 2.
      L1: hT_rep[120,512] = w1rep_chunk.T @ xT_chunk  (3 chunk matmuls)
      DVE: h = relu(hT_rep) cast to bf16
      L2: tT_rep[125,512] = w2rep.T @ h[0:10]         (K=10)
      ACT: tT_all[:, cols] = relu(tT_rep + b2rep)     (bias per-partition)
    tT_all [125, 25088] f32 lives entirely in SBUF (~98KB/partition) - no
    DRAM round trip for t.
  - The size-5 window gather costs nothing: out.T[5w+c, n] = tT[c, n+w] =
    tT_all[5w+c, n+w], so each superblock issues 5 plain strided store
    DMAs (one per window shift w), 8KB per descriptor.
  - A ~6us burst of full-width matmuls on scratch data warms the HAM
    clock-gate while the first x loads stream in (PE is idle then anyway).
  - x loads stream on the SP HWDGE ring (3 DMAs of 512KB per superblock,
    triple buffered); stores go on the ACT ring so they never head-of-line
    block a load.
  - Host unshard: concat the per-core outT [25, 25000] along columns,
    transpose to [200000, 25], patch the 4 global-edge window slots to
    exact zero (the reference zero-pads t, not x).
  - The ISA allows ONE sync-wait per instruction; a post-pass hoists any
    extra waits onto same-engine NoOps.
"""

import numpy as np
import ml_dtypes

import bass_rust
import concourse.bass as bass
import concourse.mybir as mybir
import concourse.tile as tile

# ---- problem constants (hardcoded per contract) ----
N = 200000
D = 320
D1 = 10
D2 = 5
W = 5
HALF = W // 2
NCORES = 8
ROWS = N // NCORES          # 25000 output rows per core
BLK = 512                   # rows per compute block (one PSUM bank)
NBLK = 49                   # 25088 padded rows of t per core
PAD = NBLK * BLK            # 25088
SBLK = 4                    # compute blocks per superblock (DMA granularity)
CHUNKS = [(0, 128), (128, 128), (256, 64)]  # d-chunks of 320
M1 = 12 * D1                # 120: W1T replicas (HAM fuel)
M2 = 25 * D2                # 125: W2T replicas (HAM fuel + port spread)
NWARM = 14                  # warmup matmuls (~6us cold) to lift HAM to 2.4GHz
F32 = mybir.dt.float32
BF16 = mybir.dt.bfloat16
RELU = mybir.ActivationFunctionType.Relu
BF = ml_dtypes.bfloat16

_NC_CACHE = {}


def split_multiwaits(nc):
    """Walrus/ISA allows ONE sync-wait per instruction; Tile emits several.

    For every instruction with >1 wait, hoist all but the last wait onto
    fresh NoOps on the same engine immediately before it.  The engine
    stalls at the nops exactly as it would have at the instruction, so
    semantics are unchanged.
    """
    n_split = 0
    for bb in nc.main_func.blocks:
        insts = bb.instructions
        out = []
        changed = False
        for ins in insts:
            si = ins.sync_info
            waits = list(si.on_wait) if si is not None else []
            if len(waits) > 1:
                changed = True
                for w in waits[:-1]:
                    n_split += 1
                    nop = bass_rust.InstNoOp(name=f"wsplit-{n_split}")
                    nop.engine = ins.engine
                    nop.sync_info = bass_rust.SyncInfo(
                        on_wait=[w], on_update=[]
                    )
                    nc.inst_map[nop.name] = nop
                    out.append(nop)
                ins.sync_info = bass_rust.SyncInfo(
                    on_wait=[waits[-1]], on_update=list(si.on_update)
                )
            out.append(ins)
        if changed:
            bb.instructions = out
    return n_split


def build_nc():
    nc = bass.Bass("TRN2", target_bir_lowering=False, debug=False)

    xT_t = nc.dram_tensor("xT", [D, PAD], BF16, kind="ExternalInput")
    w1r_t = nc.dram_tensor("W1R", [D, M1], BF16, kind="ExternalInput")
    w2r_t = nc.dram_tensor("W2R", [D1, M2], BF16, kind="ExternalInput")
    b2_t = nc.dram_tensor("b2", [D2], F32, kind="ExternalInput")
    out_t = nc.dram_tensor("outT", [W * D2, ROWS], F32, kind="ExternalOutput")

    # superblock start columns (in t rows): 12 x 2048 + 1 x 512
    sb_starts = list(range(0, PAD, SBLK * BLK))
    sb_lens = [min(SBLK * BLK, PAD - s) for s in sb_starts]
    NSB = len(sb_starts)

    with tile.TileContext(nc) as tc:
        with (
            tc.tile_pool(name="singles", bufs=1) as singles,
            tc.tile_pool(name="xpool", bufs=3) as xpool,
            tc.tile_pool(name="hpool", bufs=4) as hpool,
            tc.tile_pool(name="ps_h", bufs=2, space="PSUM") as ps_h,
            tc.tile_pool(name="ps_t", bufs=2, space="PSUM") as ps_t,
            tc.tile_pool(name="ps_w", bufs=1, space="PSUM") as ps_w,
        ):
            # ---- constants (one-time) ----
            w1r_sb = singles.tile([128, len(CHUNKS), M1], BF16)
            for c, (d0, cw) in enumerate(CHUNKS):
                nc.sync.dma_start(
                    out=w1r_sb[:cw, c, :],
                    in_=bass.AP(w1r_t, d0 * M1, [[M1, cw], [1, M1]]),
                )
            w2r_sb = singles.tile([D1, M2], BF16)
            nc.sync.dma_start(out=w2r_sb, in_=w2r_t[:, :])
            # b2 replicated to 125 partitions: b2rep[5w+c] = b2[c]
            b2r_sb = singles.tile([M2, 1], F32)
            nc.gpsimd.dma_start(
                out=b2r_sb, in_=bass.AP(b2_t, 0, [[0, M2 // D2], [1, D2]])
            )
            # persistent t.T accumulator [125, 25088] f32 (~98KB/partition)
            tT_all = singles.tile([M2, PAD], F32)

            # ---- HAM warmup: full-width matmuls on scratch while the
            # first x loads stream in (PE is otherwise idle) ----
            warm_sb = singles.tile([128, BLK], BF16)
            nc.vector.memset(warm_sb, 0.625)
            warm_ps = ps_w.tile([128, BLK], F32, tag="w")
            for i in range(NWARM):
                nc.tensor.matmul(
                    warm_ps, warm_sb[:, :128], warm_sb,
                    start=True, stop=True,
                )

            x_sbs = {}      # sb index -> list of 3 chunk tiles
            h_sbs = {}      # block index -> h tile [120, 512] bf16
            t_pss = {}      # block index -> tT psum tile [125, 512]

            def emit_loads(s):
                tiles = []
                for c, (d0, cw) in enumerate(CHUNKS):
                    xt = xpool.tile([128, SBLK * BLK], BF16, tag=f"x{c}")
                    nc.sync.dma_start(
                        out=xt[:cw, : sb_lens[s]],
                        in_=bass.AP(
                            xT_t,
                            d0 * PAD + sb_starts[s],
                            [[PAD, cw], [1, sb_lens[s]]],
                        ),
                    )
                    tiles.append(xt)
                x_sbs[s] = tiles

            def emit_l1(b):
                """3 chunk matmuls + DVE relu for block b."""
                s, r = divmod(b, SBLK)
                h_ps = ps_h.tile([M1, BLK], F32, tag="h")
                for c, (d0, cw) in enumerate(CHUNKS):
                    nc.tensor.matmul(
                        h_ps,
                        w1r_sb[:cw, c, :],
                        x_sbs[s][c][:cw, r * BLK : (r + 1) * BLK],
                        start=(c == 0),
                        stop=(c == len(CHUNKS) - 1),
                    )
                h_sb = hpool.tile([M1, BLK], BF16, tag="hs")
                nc.vector.tensor_scalar_max(h_sb, h_ps, 0.0)
                h_sbs[b] = h_sb

            def emit_l2(b):
                """L2 matmul for block b (lagged one block so the PE never
                stalls on a fresh DVE relu)."""
                t_ps = ps_t.tile([M2, BLK], F32, tag="t")
                nc.tensor.matmul(
                    t_ps, w2r_sb, h_sbs[b][:D1, :], start=True, stop=True
                )
                t_pss[b] = t_ps

            def emit_bias_relu(b):
                """ACT: tT_all[:, block cols] = relu(tT_ps + b2rep)."""
                nc.scalar.activation(
                    tT_all[:, b * BLK : (b + 1) * BLK],
                    t_pss[b],
                    RELU,
                    bias=b2r_sb,
                )
                del t_pss[b]

            def emit_store(s):
                """outT[5w+c, n] = tT[c, n+w] = tT_all[5w+c, n+w]: one
                plain strided store per window shift w."""
                n0 = sb_starts[s]
                ln = min(sb_lens[s], ROWS - n0)
                for w in range(W):
                    nc.scalar.dma_start(
                        out=bass.AP(
                            out_t,
                            w * D2 * ROWS + n0,
                            [[ROWS, D2], [1, ln]],
                        ),
                        in_=tT_all[w * D2 : (w + 1) * D2, n0 + w : n0 + w + ln],
                    )

            # ---- main loop (software-pipelined) ----
            emit_loads(0)
            emit_loads(1)
            for b in range(NBLK):
                s, r = divmod(b, SBLK)
                if r == 0 and s + 2 < NSB:
                    emit_loads(s + 2)
                emit_l1(b)
                if b >= 1:
                    emit_l2(b - 1)
                if b >= 2:
                    emit_bias_relu(b - 2)
                # store superblock s-1 once its +4 halo cols exist
                # (after bias_relu of block 4s, i.e. when b-2 == 4s)
                if r == 2 and s >= 1:
                    emit_store(s - 1)

            emit_l2(NBLK - 1)
            emit_bias_relu(NBLK - 2)
            emit_bias_relu(NBLK - 1)
            emit_store(NSB - 2)
            emit_store(NSB - 1)

    split_multiwaits(nc)
    return nc


def make_shards(x):
    """Per-core xT [320, PAD] bf16 shards with +-2 col halo, zero padded."""
    xbT = np.ascontiguousarray(x.astype(BF).T)  # [320, N]
    shards = []
    for c in range(NCORES):
        s = np.zeros((D, PAD), dtype=BF)
        lo = ROWS * c - HALF
        src_lo, src_hi = max(lo, 0), min(lo + PAD, N)
        s[:, src_lo - lo : src_lo - lo + (src_hi - src_lo)] = xbT[
            :, src_lo:src_hi
        ]
        shards.append(s)
    return shards


def _patch_edges(out):
    # the reference zero-pads t, not x: window slots that fall outside
    # [0, N) must be exactly zero.
    out[0, : 2 * D2] = 0.0
    out[1, :D2] = 0.0
    out[N - 2, 4 * D2 :] = 0.0
    out[N - 1, 3 * D2 :] = 0.0
    return out


def run(inputs, trace=False):
    from concourse.bass_utils import run_bass_kernel_spmd

    x = np.ascontiguousarray(np.asarray(inputs["x"], dtype=np.float32))
    W1 = np.asarray(inputs["W1"], dtype=np.float32)
    W2 = np.asarray(inputs["W2"], dtype=np.float32)
    b2 = np.ascontiguousarray(np.asarray(inputs["b2"], dtype=np.float32))
    assert x.shape == (N, D)

    W1R = np.ascontiguousarray(np.tile(W1.T, (1, M1 // D1))).astype(BF)
    W2R = np.ascontiguousarray(np.tile(W2.T, (1, M2 // D2))).astype(BF)

    if "nc" not in _NC_CACHE:
        _NC_CACHE["nc"] = build_nc()
    nc = _NC_CACHE["nc"]

    in_maps = [
        {"xT": s, "W1R": W1R, "W2R": W2R, "b2": b2} for s in make_shards(x)
    ]
    res = run_bass_kernel_spmd(nc, in_maps, list(range(NCORES)), trace=trace)
    out = np.ascontiguousarray(
        np.concatenate(
            [res.results[c]["outT"] for c in range(NCORES)], axis=1
        ).T
    )
    return _patch_edges(out), res


def kernel(**inputs):
    out, _ = run(inputs, trace=False)
    return out
